# revision 1
# baseline (speedup 1.0000x reference)
"""Trainium2 Bass kernel for nn_DeformableDynamicGather1D.

Sharding: 8 cores = 4 batches x 2 query-halves. Each core handles one batch's
feat [256, 4096] and Q=4096 queries. Per core:

  1. Transpose feat [C, L] -> feat_T [L, C] in DRAM (PE transposes, one
     staging buffer, ONE store DMA so downstream gathers have few sem waits).
  2. Anchor: bilinear indices from coords; dma_gather 2KB row-pairs
     (rows i0, i0+1 = 512 floats, elem_step=256) query-major; lerp on DVE;
     PE-transpose into channel-major rinT for the MLP.
  3. MLP on PE: h = leaky(rin@W1+b1); g = leaky(h@(Wr+I)+br);
     out3 = [g;1]@[W3;b3] per 128-query chunk (residual folded into Wr+I,
     b3 folded via augmented ones row).
  4. Scalar stage (query-major [128, 32] tiles): softplus/clips, tanh,
     sigmoid, offsets, deform indices, normalized bilinear weights c0/c1.
  5. Deform: dma_gather 5 taps x 4 chunks; accumulate with
     scalar_tensor_tensor FMAs into ob [128, 32, 256]; one 4MB out DMA.

Query <-> tile coordinates: q = g*128 + p (tile [128 p, 32 g]); dma_gather
places index-list position j at out [j%128, j//128] and reads idx j from a
wrapped int16 tile at [j%16, j//16] (16-row block replicated on all 128
partitions for the 8 Q7 cores). With j = q, the wrapped tile w[b, f] =
i0(q=16f+b) is built from the query-major f32 index tile V [128, (g,k)] by
8 constant selection matmuls W_a[b, n] = V[16a+b, n] (PE does the partition
fold), strided copies (col f = g*8 + a), int16 convert, and one 8x partition
replication DMA.
"""
import os
import sys

for _p in ("/opt/trn_rl_repo", "/root/.axon_site/_ro/trn_rl_repo"):
    if os.path.isdir(_p) and _p not in sys.path:
        sys.path.append(_p)

import numpy as np
import concourse.bass as bass
import concourse.bacc as bacc
import concourse.tile as tile
from concourse import mybir
from concourse.bass import AP
from concourse.masks import make_identity

F32 = mybir.dt.float32
I16 = mybir.dt.int16
I32 = mybir.dt.int32
Act = mybir.ActivationFunctionType
Alu = mybir.AluOpType

P = 128          # partitions
G = 32           # q = g*128 + p
Q = P * G        # 4096 queries per core
C = 256          # channels
L = 4096         # feat length
H = 64           # hidden
K = 5            # taps
NCORES = 8
B, N = 4, 8192   # full problem
NI = 1024        # idxs per dma_gather call
NCH = Q // NI    # 4 chunks
GPC = NI // P    # 8 g-columns per chunk

IXSCALE = np.float32(float(L - 1))          # 4095
DXSCALE = np.float32(2.0 / max(L - 1, 1))   # reference scale_x

DEBUG_DUMPS = False


def _bc(ap2d: AP, extra: int) -> AP:
    """Broadcast a [p, n] AP to [p, n, extra] with stride-0 inner dim."""
    return AP(tensor=ap2d.tensor, offset=ap2d.offset,
              ap=[*ap2d.ap, [0, extra]])


def _bc_mid(ap2d: AP, mid: int) -> AP:
    """Broadcast a [p, n] AP to [p, mid, n] with stride-0 middle dim."""
    return AP(tensor=ap2d.tensor, offset=ap2d.offset,
              ap=[ap2d.ap[0], [0, mid], ap2d.ap[1]])


def build_program():
    nc = bacc.Bacc("TRN2", target_bir_lowering=False, debug=False,
                   num_devices=NCORES)

    feat = nc.dram_tensor("feat", [C, L], F32, kind="ExternalInput")
    coords = nc.dram_tensor("coords", [Q], F32, kind="ExternalInput")
    cellv = nc.dram_tensor("cellv", [Q], F32, kind="ExternalInput")
    w1a0 = nc.dram_tensor("w1a0", [128, H], F32, kind="ExternalInput")
    w1a1 = nc.dram_tensor("w1a1", [128, H], F32, kind="ExternalInput")
    wxc = nc.dram_tensor("wxc", [2, H], F32, kind="ExternalInput")
    b1c = nc.dram_tensor("b1c", [H, 1], F32, kind="ExternalInput")
    wr1 = nc.dram_tensor("wr1", [H, H], F32, kind="ExternalInput")
    brc = nc.dram_tensor("brc", [H, 1], F32, kind="ExternalInput")
    w3aug = nc.dram_tensor("w3aug", [H + 1, 12], F32, kind="ExternalInput")
    base128 = nc.dram_tensor("base128", [P, K], F32, kind="ExternalInput")
    sel8 = nc.dram_tensor("sel8", [P, 8 * 128], F32, kind="ExternalInput")
    out = nc.dram_tensor("out", [Q, C], F32, kind="ExternalOutput")

    dbg = {}
    if DEBUG_DUMPS:
        dbg = {
            "d_featT": nc.dram_tensor("d_featT", [L, C], F32, kind="ExternalOutput"),
            "d_aidx": nc.dram_tensor("d_aidx", [P, G], F32, kind="ExternalOutput"),
            "d_wrapA": nc.dram_tensor("d_wrapA", [P, Q // 16], I16, kind="ExternalOutput"),
            "d_Ga0": nc.dram_tensor("d_Ga0", [P, GPC * 512], F32, kind="ExternalOutput"),
            "d_rinT0": nc.dram_tensor("d_rinT0", [P, Q], F32, kind="ExternalOutput"),
            "d_out3": nc.dram_tensor("d_out3", [P, G * 12], F32, kind="ExternalOutput"),
            "d_didx": nc.dram_tensor("d_didx", [P, G * K], F32, kind="ExternalOutput"),
            "d_c0": nc.dram_tensor("d_c0", [P, G * K], F32, kind="ExternalOutput"),
            "d_c1": nc.dram_tensor("d_c1", [P, G * K], F32, kind="ExternalOutput"),
            "d_Gd0": nc.dram_tensor("d_Gd0", [P, GPC * 512], F32, kind="ExternalOutput"),
        }

    with tile.TileContext(nc) as tc:
        _body(nc, tc, feat, coords, cellv, w1a0, w1a1, wxc, b1c, wr1, brc,
              w3aug, base128, sel8, out, dbg)
    nc.compile()
    return nc


def _body(nc, tc, feat, coords, cellv, w1a0, w1a1, wxc, b1c, wr1, brc,
          w3aug, base128, sel8, out, dbg=None):
    dbg = dbg or {}
    import contextlib
    ctx = contextlib.ExitStack()
    with ctx:
        persist = ctx.enter_context(tc.tile_pool(name="persist", bufs=1))
        small = ctx.enter_context(tc.tile_pool(name="small", bufs=1))
        tbuf = ctx.enter_context(tc.tile_pool(name="tbuf", bufs=2))
        gath = ctx.enter_context(tc.tile_pool(name="gath", bufs=2))
        fabuf = ctx.enter_context(tc.tile_pool(name="fabuf", bufs=3))
        big32 = ctx.enter_context(tc.tile_pool(name="big32", bufs=1))
        pst = ctx.enter_context(tc.tile_pool(name="pst", bufs=2, space="PSUM"))
        psmm = ctx.enter_context(tc.tile_pool(name="psmm", bufs=2, space="PSUM"))
        psl3 = ctx.enter_context(tc.tile_pool(name="psl3", bufs=2, space="PSUM"))
        dram = ctx.enter_context(tc.tile_pool(name="dram", bufs=1, space="DRAM"))

        ident = small.tile([P, P], F32)
        make_identity(nc, ident[:])

        feat_T = dram.tile([L, C], F32)
        rinT0 = persist.tile([P, Q], F32)      # channels 0..127, col = q
        rinT1 = persist.tile([P, Q], F32)      # channels 128..255
        xc = persist.tile([2, Q], F32)         # rows: coords, cell (q-contig)
        h_sb = persist.tile([H, Q], F32)
        gaug = persist.tile([H + 1, Q], F32)   # row H = 1.0 (b3 fold)
        out3 = persist.tile([P, G, 12], F32)

        # weights / constants
        w1a0_sb = small.tile([128, H], F32)
        w1a1_sb = small.tile([128, H], F32)
        wxc_sb = small.tile([2, H], F32)
        b1_sb = small.tile([H, 1], F32)
        wr1_sb = small.tile([H, H], F32)
        br_sb = small.tile([H, 1], F32)
        w3_sb = small.tile([H + 1, 12], F32)
        base_sb = small.tile([P, K], F32)
        sel_sb = small.tile([P, 8 * 128], F32)
        for dst, src in ((w1a0_sb, w1a0), (w1a1_sb, w1a1), (wxc_sb, wxc),
                         (b1_sb, b1c), (wr1_sb, wr1), (br_sb, brc),
                         (w3_sb, w3aug), (base_sb, base128), (sel_sb, sel8)):
            nc.sync.dma_start(out=dst[:], in_=src.ap())

        # feat_T row-pair view for dma_gather: row i = elems [256*i, 256*i+512)
        gsrc = AP(tensor=feat_T[:].tensor, offset=0,
                  ap=[[C, L - 1], [1, 2 * C]])

        def wrapped_idx(vf32_ap, nk, tag):
            """Build replicated wrapped int16 idx tile from a query-major f32
            index tile V [128, nk*G] ((g, k)-major cols: n = g*nk + k).
            SEL_a[pp, m] = (pp == 16a + m%16), so the matmul output
            W_a[m, n] = V[16a + m%16, n] is the a-th 16-row block already
            replicated on all 128 partitions. Returns [128, nk*(Q//16)] int16;
            tap k occupies cols [k*(Q//16), (k+1)*(Q//16)), col f = j//16."""
            wrep = small.tile([P, nk, Q // 16], I16, tag=tag + "_wrep")
            for a in range(8):
                psw = psl3.tile([P, nk * G], F32, tag="pswrap", space="PSUM")
                nc.tensor.matmul(
                    out=psw[:], lhsT=sel_sb[:, a * 128:(a + 1) * 128],
                    rhs=vf32_ap, start=True, stop=True)
                # psw[b, g*nk + k] -> wrep[b, k, g*8 + a]
                dst = AP(tensor=wrep[:].tensor, offset=wrep[:].offset + a,
                         ap=[wrep[:].ap[0], [Q // 16, nk], [8, G]])
                src = AP(tensor=psw[:].tensor, offset=psw[:].offset,
                         ap=[psw[:].ap[0], [1, nk], [nk, G]])
                nc.vector.tensor_copy(out=dst, in_=src)
            return wrep

        # =========== Phase T: feat [C, L] -> feat_T [L, C] ===========
        stag = big32.tile([P, G, C], F32, tag="big32")
        for t8 in range(8):
            for hh in range(2):
                ft = tbuf.tile([P, 512], F32, tag="ftin")
                nc.sync.dma_start(
                    out=ft[:],
                    in_=feat.ap()[hh * 128:(hh + 1) * 128,
                                  t8 * 512:(t8 + 1) * 512])
                for s in range(4):
                    tp = pst.tile([P, P], F32, tag="tpsum", space="PSUM")
                    nc.tensor.transpose(out=tp[:],
                                        in_=ft[:, s * 128:(s + 1) * 128],
                                        identity=ident[:])
                    nc.scalar.copy(out=stag[:, t8 * 4 + s, hh * 128:(hh + 1) * 128],
                                   in_=tp[:])
        nc.sync.dma_start(
            out=feat_T[:].rearrange("(t p) c -> p t c", p=P), in_=stag[:])
        if "d_featT" in dbg:
            rb = gath.tile([P, G // 2, C], F32, tag="gath")
            for half in range(2):
                nc.sync.dma_start(
                    out=rb[:],
                    in_=feat_T[half * 2048:(half + 1) * 2048, :].rearrange(
                        "(t p) c -> p t c", p=P))
                nc.sync.dma_start(
                    out=dbg["d_featT"].ap()[half * 2048:(half + 1) * 2048, :]
                    .rearrange("(t p) c -> p t c", p=P),
                    in_=rb[:])

        # =========== Phase A: coords, anchor idx, gather, rinT ==========
        # xq[p, g] = coords[g*128 + p]
        xq = persist.tile([P, G], F32)
        nc.sync.dma_start(
            out=xq[:],
            in_=AP(tensor=coords.ap().tensor, offset=0, ap=[[1, P], [P, G]]))
        nc.sync.dma_start(out=xc[0:1, :], in_=coords.ap().rearrange(
            "(a q) -> a q", a=1))
        nc.sync.dma_start(out=xc[1:2, :], in_=cellv.ap().rearrange(
            "(a q) -> a q", a=1))

        # ix = clip(((x + 1) * 0.5) * (L-1), 0, L-1)  (same op order as ref)
        ixf = persist.tile([P, G], F32)
        nc.vector.tensor_scalar(out=ixf[:], in0=xq[:], scalar1=1.0,
                                scalar2=0.5, op0=Alu.add, op1=Alu.mult)
        nc.vector.tensor_scalar(out=ixf[:], in0=ixf[:], scalar1=float(IXSCALE),
                                scalar2=0.0, op0=Alu.mult, op1=Alu.max)
        nc.vector.tensor_scalar(out=ixf[:], in0=ixf[:], scalar1=float(IXSCALE),
                                scalar2=None, op0=Alu.min)
        # i0 = min(floor(ix), L-2); frac = ix - i0 (identical bilinear result;
        # floor via int-convert + fixup, works for trunc or round-nearest)
        fraca = persist.tile([P, G], F32)
        i0fa = small.tile([P, G], F32)
        ti_a = small.tile([P, G], I32)
        nc.vector.tensor_copy(out=ti_a[:], in_=ixf[:])
        nc.vector.tensor_copy(out=i0fa[:], in_=ti_a[:])
        gt_a = small.tile([P, G], F32)
        nc.vector.tensor_tensor(out=gt_a[:], in0=i0fa[:], in1=ixf[:],
                                op=Alu.is_gt)
        nc.vector.tensor_tensor(out=i0fa[:], in0=i0fa[:], in1=gt_a[:],
                                op=Alu.subtract)
        nc.vector.tensor_scalar(out=i0fa[:], in0=i0fa[:], scalar1=float(L - 2),
                                scalar2=None, op0=Alu.min)
        nc.vector.tensor_tensor(out=fraca[:], in0=ixf[:], in1=i0fa[:],
                                op=Alu.subtract)
        if "d_aidx" in dbg:
            nc.sync.dma_start(out=dbg["d_aidx"].ap(), in_=i0fa[:])

        wrapA = wrapped_idx(i0fa[:], 1, "wa")
        if "d_wrapA" in dbg:
            nc.sync.dma_start(out=dbg["d_wrapA"].ap(), in_=wrapA[:])

        for ch in range(NCH):
            Ga = gath.tile([P, GPC, 2 * C], F32, tag="gath")
            nc.gpsimd.dma_gather(
                out_ap=Ga[:], in_ap=gsrc,
                idxs_ap=wrapA[:, 0, ch * (NI // 16):(ch + 1) * (NI // 16)],
                num_idxs=NI, num_idxs_reg=NI, elem_size=2 * C, elem_step=C)
            if ch == 0 and "d_Ga0" in dbg:
                nc.sync.dma_start(out=dbg["d_Ga0"].ap(), in_=Ga[:])
            for gi in range(GPC):
                g = ch * GPC + gi
                d = fabuf.tile([P, C], F32, tag="dlerp")
                nc.vector.tensor_tensor(out=d[:], in0=Ga[:, gi, 256:512],
                                        in1=Ga[:, gi, 0:256], op=Alu.subtract)
                fa = fabuf.tile([P, C], F32, tag="fa")
                nc.vector.scalar_tensor_tensor(
                    out=fa[:], in0=d[:], scalar=fraca[:, g:g + 1],
                    in1=Ga[:, gi, 0:256], op0=Alu.mult, op1=Alu.add)
                for hh in range(2):
                    tpa = pst.tile([P, P], F32, tag="tpsum", space="PSUM")
                    nc.tensor.transpose(out=tpa[:],
                                        in_=fa[:, hh * 128:(hh + 1) * 128],
                                        identity=ident[:])
                    rdst = (rinT0 if hh == 0 else rinT1)
                    nc.scalar.copy(out=rdst[:, g * 128:(g + 1) * 128],
                                   in_=tpa[:])
        if "d_rinT0" in dbg:
            nc.sync.dma_start(out=dbg["d_rinT0"].ap(), in_=rinT0[:])

        # =========== Phase M: MLP ===========
        nc.vector.memset(gaug[H:H + 1, :], 1.0)
        for n in range(8):
            sl = slice(n * 512, (n + 1) * 512)
            ps1 = psmm.tile([H, 512], F32, tag="ps1", space="PSUM")
            nc.tensor.matmul(out=ps1[:], lhsT=w1a0_sb[:], rhs=rinT0[:, sl],
                             start=True, stop=False)
            nc.tensor.matmul(out=ps1[:], lhsT=w1a1_sb[:], rhs=rinT1[:, sl],
                             start=False, stop=False)
            nc.tensor.matmul(out=ps1[:], lhsT=wxc_sb[:], rhs=xc[:, sl],
                             start=False, stop=True)
            tmp = fabuf.tile([H, 512], F32, tag="mlptmp")
            nc.scalar.activation(out=tmp[:], in_=ps1[:], func=Act.Identity,
                                 bias=b1_sb[:, :], scale=1.0)
            nc.vector.scalar_tensor_tensor(out=h_sb[:, sl], in0=tmp[:],
                                           scalar=0.2, in1=tmp[:],
                                           op0=Alu.mult, op1=Alu.max)
        for n in range(8):
            sl = slice(n * 512, (n + 1) * 512)
            ps2 = psmm.tile([H, 512], F32, tag="ps1", space="PSUM")
            nc.tensor.matmul(out=ps2[:], lhsT=wr1_sb[:], rhs=h_sb[:, sl],
                             start=True, stop=True)
            tmp2 = fabuf.tile([H, 512], F32, tag="mlptmp")
            nc.scalar.activation(out=tmp2[:], in_=ps2[:], func=Act.Identity,
                                 bias=br_sb[:, :], scale=1.0)
            nc.vector.scalar_tensor_tensor(out=gaug[0:H, sl], in0=tmp2[:],
                                           scalar=0.2, in1=tmp2[:],
                                           op0=Alu.mult, op1=Alu.max)
        for g in range(G):
            ps3 = psl3.tile([P, 12], F32, tag="ps3", space="PSUM")
            nc.tensor.matmul(out=ps3[:], lhsT=gaug[:, g * 128:(g + 1) * 128],
                             rhs=w3_sb[:], start=True, stop=True)
            nc.scalar.copy(out=out3[:, g, :], in_=ps3[:])
        if "d_out3" in dbg:
            nc.sync.dma_start(out=dbg["d_out3"].ap(), in_=out3[:])

        # =========== Phase S: scalar stage ===========
        sc = ctx.enter_context(tc.tile_pool(name="scal", bufs=1))

        def softplus(dst, src_ap):
            a = sc.tile([P, G], F32, tag="sp_a")
            nc.scalar.activation(out=a[:], in_=src_ap, func=Act.Abs)
            e = sc.tile([P, G], F32, tag="sp_e")
            nc.scalar.activation(out=e[:], in_=a[:], func=Act.Exp, scale=-1.0)
            lg = sc.tile([P, G], F32, tag="sp_l")
            nc.scalar.activation(out=lg[:], in_=e[:], func=Act.Ln, bias=1.0,
                                 scale=1.0)
            m = sc.tile([P, G], F32, tag="sp_m")
            nc.vector.tensor_scalar(out=m[:], in0=src_ap, scalar1=0.0,
                                    scalar2=None, op0=Alu.max)
            nc.vector.tensor_tensor(out=dst, in0=lg[:], in1=m[:], op=Alu.add)

        r_t = sc.tile([P, G], F32)
        softplus(r_t[:], out3[:, :, 0])
        nc.vector.tensor_scalar(out=r_t[:], in0=r_t[:], scalar1=0.3,
                                scalar2=2.0, op0=Alu.add, op1=Alu.min)
        sg_t = sc.tile([P, G], F32)
        softplus(sg_t[:], out3[:, :, 1])
        nc.vector.tensor_scalar(out=sg_t[:], in0=sg_t[:], scalar1=0.5,
                                scalar2=3.0, op0=Alu.add, op1=Alu.min)
        s2 = sc.tile([P, G], F32)
        nc.vector.tensor_tensor(out=s2[:], in0=sg_t[:], in1=sg_t[:],
                                op=Alu.mult)
        nc.vector.tensor_scalar(out=s2[:], in0=s2[:], scalar1=4.0,
                                scalar2=1e-8, op0=Alu.mult, op1=Alu.add)
        rs = sc.tile([P, G], F32)
        nc.vector.reciprocal(out=rs[:], in_=s2[:])

        res_t = sc.tile([P, G * K], F32)
        nc.scalar.activation(out=res_t[:], in_=out3[:, :, 2:7], func=Act.Tanh)
        gate_t = sc.tile([P, G * K], F32)
        nc.scalar.activation(out=gate_t[:], in_=out3[:, :, 7:12],
                             func=Act.Sigmoid)

        off_t = sc.tile([P, G * K], F32)
        nc.vector.tensor_tensor(out=off_t[:], in0=_bc(r_t[:], K),
                                in1=_bc_mid(base_sb[:], G), op=Alu.mult)
        nc.vector.scalar_tensor_tensor(out=off_t[:], in0=res_t[:], scalar=0.5,
                                       in1=off_t[:], op0=Alu.mult, op1=Alu.add)
        dix = sc.tile([P, G * K], F32)
        nc.vector.scalar_tensor_tensor(out=dix[:], in0=off_t[:],
                                       scalar=float(DXSCALE),
                                       in1=_bc(xq[:], K),
                                       op0=Alu.mult, op1=Alu.add)
        nc.vector.tensor_scalar(out=dix[:], in0=dix[:], scalar1=1.0,
                                scalar2=0.5, op0=Alu.add, op1=Alu.mult)
        nc.vector.tensor_scalar(out=dix[:], in0=dix[:], scalar1=float(IXSCALE),
                                scalar2=0.0, op0=Alu.mult, op1=Alu.max)
        nc.vector.tensor_scalar(out=dix[:], in0=dix[:], scalar1=float(IXSCALE),
                                scalar2=None, op0=Alu.min)
        fracd = sc.tile([P, G * K], F32)
        i0fd = sc.tile([P, G * K], F32)
        ti_d = sc.tile([P, G * K], I32)
        nc.vector.tensor_copy(out=ti_d[:], in_=dix[:])
        nc.vector.tensor_copy(out=i0fd[:], in_=ti_d[:])
        gt_d = sc.tile([P, G * K], F32)
        nc.vector.tensor_tensor(out=gt_d[:], in0=i0fd[:], in1=dix[:],
                                op=Alu.is_gt)
        nc.vector.tensor_tensor(out=i0fd[:], in0=i0fd[:], in1=gt_d[:],
                                op=Alu.subtract)
        nc.vector.tensor_scalar(out=i0fd[:], in0=i0fd[:], scalar1=float(L - 2),
                                scalar2=None, op0=Alu.min)
        nc.vector.tensor_tensor(out=fracd[:], in0=dix[:], in1=i0fd[:],
                                op=Alu.subtract)

        o2 = sc.tile([P, G * K], F32)
        nc.vector.tensor_tensor(out=o2[:], in0=off_t[:], in1=off_t[:],
                                op=Alu.mult)
        nc.vector.tensor_tensor(out=o2[:], in0=o2[:], in1=_bc(rs[:], K),
                                op=Alu.mult)
        w_t = sc.tile([P, G * K], F32)
        nc.scalar.activation(out=w_t[:], in_=o2[:], func=Act.Exp, scale=-0.5)
        nc.vector.tensor_tensor(out=w_t[:], in0=w_t[:], in1=gate_t[:],
                                op=Alu.mult)
        wsum = sc.tile([P, G], F32)
        w_v = w_t[:].rearrange("p (g k) -> p g k", k=K)
        nc.vector.tensor_reduce(out=wsum[:], in_=w_v, axis=mybir.AxisListType.X,
                                op=Alu.add)
        nc.vector.tensor_scalar(out=wsum[:], in0=wsum[:], scalar1=1e-8,
                                scalar2=None, op0=Alu.add)
        rn = sc.tile([P, G], F32)
        nc.vector.reciprocal(out=rn[:], in_=wsum[:])
        wn = sc.tile([P, G * K], F32)
        nc.vector.tensor_tensor(out=wn[:], in0=w_t[:], in1=_bc(rn[:], K),
                                op=Alu.mult)
        c1 = sc.tile([P, G * K], F32)
        nc.vector.tensor_tensor(out=c1[:], in0=wn[:], in1=fracd[:],
                                op=Alu.mult)
        c0 = sc.tile([P, G * K], F32)
        nc.vector.tensor_tensor(out=c0[:], in0=wn[:], in1=c1[:],
                                op=Alu.subtract)
        if "d_didx" in dbg:
            nc.sync.dma_start(out=dbg["d_didx"].ap(), in_=i0fd[:])
            nc.sync.dma_start(out=dbg["d_c0"].ap(), in_=c0[:])
            nc.sync.dma_start(out=dbg["d_c1"].ap(), in_=c1[:])

        wrapD = wrapped_idx(i0fd[:], K, "wd")

        # =========== Phase G: deform gather + combine + out ===========
        ob = big32.tile([P, G, C], F32, tag="big32")
        for k in range(K):
            for ch in range(NCH):
                Gd = gath.tile([P, GPC, 2 * C], F32, tag="gath")
                nc.gpsimd.dma_gather(
                    out_ap=Gd[:], in_ap=gsrc,
                    idxs_ap=wrapD[:, k, ch * (NI // 16):(ch + 1) * (NI // 16)],
                    num_idxs=NI, num_idxs_reg=NI, elem_size=2 * C, elem_step=C)
                if k == 0 and ch == 0 and "d_Gd0" in dbg:
                    nc.sync.dma_start(out=dbg["d_Gd0"].ap(), in_=Gd[:])
                for gi in range(GPC):
                    g = ch * GPC + gi
                    acc = ob[:, g, :]
                    if k == 0:
                        nc.vector.tensor_scalar(
                            out=acc, in0=Gd[:, gi, 0:256],
                            scalar1=c0[:, g * K + k:g * K + k + 1],
                            scalar2=None, op0=Alu.mult)
                    else:
                        nc.vector.scalar_tensor_tensor(
                            out=acc, in0=Gd[:, gi, 0:256],
                            scalar=c0[:, g * K + k:g * K + k + 1],
                            in1=acc, op0=Alu.mult, op1=Alu.add)
                    nc.vector.scalar_tensor_tensor(
                        out=acc, in0=Gd[:, gi, 256:512],
                        scalar=c1[:, g * K + k:g * K + k + 1],
                        in1=acc, op0=Alu.mult, op1=Alu.add)
        nc.sync.dma_start(
            out=out.ap().rearrange("(g p) c -> p g c", p=P), in_=ob[:])


_PROGRAM = None


def _get_program():
    global _PROGRAM
    if _PROGRAM is None:
        _PROGRAM = build_program()
    return _PROGRAM


def make_in_maps(feat_1d, coords_1d, cell_1d, W1, b1, Wr, br, W3, b3):
    """Build the 8 per-core input dicts from full inputs."""
    f32 = np.float32
    W1 = np.asarray(W1, f32)
    wr1 = np.asarray(Wr, f32) + np.eye(H, dtype=f32)
    w3aug = np.concatenate([np.asarray(W3, f32),
                            np.asarray(b3, f32).reshape(1, 12)], axis=0)
    base = np.array([-2.0, -1.0, 0.0, 1.0, 2.0], f32)
    base128 = np.broadcast_to(base, (P, K)).copy()
    sel = np.zeros((P, 8, 128), f32)
    for a in range(8):
        for m in range(128):
            sel[16 * a + m % 16, a, m] = 1.0
    shared = {
        "w1a0": np.ascontiguousarray(W1[0:128]),
        "w1a1": np.ascontiguousarray(W1[128:256]),
        "wxc": np.ascontiguousarray(W1[256:258]),
        "b1c": np.asarray(b1, f32).reshape(H, 1).copy(),
        "wr1": wr1,
        "brc": np.asarray(br, f32).reshape(H, 1).copy(),
        "w3aug": w3aug,
        "base128": base128,
        "sel8": sel.reshape(P, 8 * 128),
    }
    in_maps = []
    for core in range(NCORES):
        b = core // 2
        s = core % 2
        sl = slice(s * Q, (s + 1) * Q)
        in_maps.append({
            "feat": np.ascontiguousarray(np.asarray(feat_1d[b], f32)),
            "coords": np.ascontiguousarray(np.asarray(coords_1d[b, sl, 0], f32)),
            "cellv": np.ascontiguousarray(np.asarray(cell_1d[b, sl, 0], f32)),
            **shared,
        })
    return in_maps


def kernel(feat_1d, coords_1d, cell_1d, W1, b1, Wr, br, W3, b3):
    from concourse.bass_utils import run_bass_kernel_spmd
    nc = _get_program()
    in_maps = make_in_maps(feat_1d, coords_1d, cell_1d, W1, b1, Wr, br, W3, b3)
    res = run_bass_kernel_spmd(nc, in_maps, core_ids=list(range(NCORES)))
    outf = np.zeros((B, N, C), np.float32)
    for core in range(NCORES):
        b = core // 2
        s = core % 2
        outf[b, s * Q:(s + 1) * Q, :] = res.results[core]["out"]
    return outf



# revision 10
# speedup vs baseline: 1.3561x; 1.3561x over previous
"""Trainium2 Bass kernel for nn_DeformableDynamicGather1D (v2).

Sharding: 8 cores = 4 batches x 2 query-halves; per core feat [256, 4096],
Q=4096 queries.

Key ideas vs v1 baseline:
  1. Anchor path folds layer-1 of the router MLP into a precomputed table
     U[l, h] = sum_c feat[c, l] * W1[c, h]  ([L, 64], f32 in DRAM, computed
     on PE straight from the channel-major feat with NO transpose).  The
     anchor gather then fetches 512B row-pairs of U instead of 2KB row-pairs
     of feat (4x less traffic) and the expensive 256-channel query->channel
     transposes disappear: lerp produces h_pre query-major [q, 64], a cheap
     PE transpose + a small accumulated matmul adds the coords/cell/b1
     contribution.
  2. feat_T for the deform gather is stored in fp16: deform gather traffic
     drops 40MB -> 20MB per core; DVE tap-combine runs on packed fp16
     (2x_1P mode eligible).  Output is written fp16 and cast to f32 on host.
  3. MLP runs in fp16 on the PE (1 cycle/row vs 4 for fp32) with biases
     folded into augmented weights (ones rows).
  4. Chunked software pipeline: queries processed in 4 chunks of 1024;
     chunk ch+1's anchor/MLP/scalar front is issued before chunk ch's
     deform gather+combine, so GpSimd gather transfers, DVE tap-FMAs and
     PE front work overlap across chunks.
  5. PSUM matmul groups are batched per 2KB bank (8 transposes or 8 U
     matmuls per accumulation group) to amortize PSUM->SBUF copies.
"""
import os
import sys

for _p in ("/opt/trn_rl_repo", "/root/.axon_site/_ro/trn_rl_repo"):
    if os.path.isdir(_p) and _p not in sys.path:
        sys.path.append(_p)

import numpy as np
import concourse.bass as bass
import concourse.bacc as bacc
import concourse.tile as tile
from concourse import mybir
from concourse.bass import AP
from concourse.masks import make_identity

F32 = mybir.dt.float32
F16 = mybir.dt.float16
I16 = mybir.dt.int16
I32 = mybir.dt.int32
Act = mybir.ActivationFunctionType
Alu = mybir.AluOpType

P = 128          # partitions
G = 32           # q = g*128 + p
Q = P * G        # 4096 queries per core
C = 256          # channels
L = 4096         # feat length
H = 64           # hidden
K = 5            # taps
NCORES = 8
B, N = 4, 8192   # full problem
NCH = 4          # query chunks
GC = G // NCH    # 8 g-columns per chunk
QC = P * GC      # 1024 queries per chunk
GH = GC // 2     # 4 g-columns per half-chunk
QH = P * GH      # 512 queries per half-chunk

IXSCALE = np.float32(float(L - 1))          # 4095
DXSCALE = np.float32(2.0 / max(L - 1, 1))   # reference scale_x

DEBUG_DUMPS = False


def _bc(ap2d: AP, extra: int) -> AP:
    """Broadcast a [p, n] AP to [p, n, extra] with stride-0 inner dim."""
    return AP(tensor=ap2d.tensor, offset=ap2d.offset,
              ap=[*ap2d.ap, [0, extra]])


def _bc_mid(ap2d: AP, mid: int) -> AP:
    """Broadcast a [p, n] AP to [p, mid, n] with stride-0 middle dim."""
    return AP(tensor=ap2d.tensor, offset=ap2d.offset,
              ap=[ap2d.ap[0], [0, mid], ap2d.ap[1]])


def build_program():
    nc = bacc.Bacc("TRN2", target_bir_lowering=False, debug=False,
                   num_devices=NCORES)

    feat = nc.dram_tensor("feat", [C, L], F32, kind="ExternalInput")
    coords = nc.dram_tensor("coords", [Q], F32, kind="ExternalInput")
    xc3h = nc.dram_tensor("xc3h", [3, Q], F16, kind="ExternalInput")
    w1a0 = nc.dram_tensor("w1a0", [128, H], F16, kind="ExternalInput")
    w1a1 = nc.dram_tensor("w1a1", [128, H], F16, kind="ExternalInput")
    wxc3 = nc.dram_tensor("wxc3", [3, H], F16, kind="ExternalInput")
    wr1aug = nc.dram_tensor("wr1aug", [H + 1, H], F16, kind="ExternalInput")
    w3aug = nc.dram_tensor("w3aug", [H + 1, 12], F16, kind="ExternalInput")
    base128 = nc.dram_tensor("base128", [P, K], F32, kind="ExternalInput")
    sel8 = nc.dram_tensor("sel8", [P, 8 * 128], F32, kind="ExternalInput")
    out = nc.dram_tensor("out", [Q, C], F16, kind="ExternalOutput")

    dbg = {}
    if DEBUG_DUMPS:
        dbg = {
            "d_U": nc.dram_tensor("d_U", [P, G * H], F32, kind="ExternalOutput"),
            "d_featT": nc.dram_tensor("d_featT", [P, G * C], F16, kind="ExternalOutput"),
            "d_h": nc.dram_tensor("d_h", [H + 1, Q], F16, kind="ExternalOutput"),
            "d_g": nc.dram_tensor("d_g", [H + 1, Q], F16, kind="ExternalOutput"),
            "d_out3": nc.dram_tensor("d_out3", [P, GC * 12], F32, kind="ExternalOutput"),
            "d_i0fd": nc.dram_tensor("d_i0fd", [P, GC * K], F32, kind="ExternalOutput"),
            "d_c0": nc.dram_tensor("d_c0", [P, GC * K], F32, kind="ExternalOutput"),
            "d_c1": nc.dram_tensor("d_c1", [P, GC * K], F32, kind="ExternalOutput"),
        }

    with tile.TileContext(nc) as tc:
        _body(nc, tc, feat, coords, xc3h, w1a0, w1a1, wxc3, wr1aug,
              w3aug, base128, sel8, out, dbg)
    nc.compile()
    return nc


def _body(nc, tc, feat, coords, xc3h, w1a0, w1a1, wxc3, wr1aug,
          w3aug, base128, sel8, out, dbg=None):
    dbg = dbg or {}
    import contextlib
    ctx = contextlib.ExitStack()
    with ctx:
        persist = ctx.enter_context(tc.tile_pool(name="persist", bufs=1))
        small = ctx.enter_context(tc.tile_pool(name="small", bufs=1))
        gathA = ctx.enter_context(tc.tile_pool(name="gathA", bufs=2))
        gathD = ctx.enter_context(tc.tile_pool(name="gathD", bufs=2))
        fap = ctx.enter_context(tc.tile_pool(name="fap", bufs=2))
        obp = ctx.enter_context(tc.tile_pool(name="obp", bufs=2))
        sc = ctx.enter_context(tc.tile_pool(name="scal", bufs=2))
        pst = ctx.enter_context(tc.tile_pool(name="pst", bufs=1, space="PSUM"))
        psU = ctx.enter_context(tc.tile_pool(name="psU", bufs=1, space="PSUM"))
        psa = ctx.enter_context(tc.tile_pool(name="psa", bufs=2, space="PSUM"))
        psb = ctx.enter_context(tc.tile_pool(name="psb", bufs=2, space="PSUM"))
        psl3 = ctx.enter_context(tc.tile_pool(name="psl3", bufs=2, space="PSUM"))
        dram = ctx.enter_context(tc.tile_pool(name="dram", bufs=1, space="DRAM"))

        # ---------------- persistent tiles ----------------
        feat32 = persist.tile([P, 2, L], F32)     # c-halves of feat
        feat16 = persist.tile([P, 2, L], F16)
        stagT = persist.tile([P, G, C], F16)      # feat_T staging (t-major)
        stagU = persist.tile([P, G, H], F32)      # U staging
        h_sb = persist.tile([H + 1, Q], F16)      # row H = 1.0
        gaug = persist.tile([H + 1, Q], F16)      # row H = 1.0
        xq = persist.tile([P, G], F32)
        ixf = persist.tile([P, G], F32)
        fraca = persist.tile([P, G], F32)
        i0fa = persist.tile([P, G], F32)

        feat_T = dram.tile([L, C], F16)
        U_dram = dram.tile([L, H], F32)

        # ---------------- weights / constants ----------------
        w1a0_sb = small.tile([128, H], F16)
        w1a1_sb = small.tile([128, H], F16)
        wxc3_sb = small.tile([3, H], F16)
        wr1_sb = small.tile([H + 1, H], F16)
        w3_sb = small.tile([H + 1, 12], F16)
        base_sb = small.tile([P, K], F32)
        sel_sb = small.tile([P, 8 * 128], F32)
        xc3_sb = small.tile([3, Q], F16)
        ident32 = small.tile([P, P], F32)
        wrapA = small.tile([P, Q // 16], I16)

        for dst, src in ((w1a0_sb, w1a0), (w1a1_sb, w1a1), (wxc3_sb, wxc3),
                         (wr1_sb, wr1aug), (w3_sb, w3aug),
                         (base_sb, base128), (sel_sb, sel8), (xc3_sb, xc3h)):
            nc.sync.dma_start(out=dst[:], in_=src.ap())
        nc.sync.dma_start(
            out=xq[:],
            in_=AP(tensor=coords.ap().tensor, offset=0, ap=[[1, P], [P, G]]))
        nc.sync.dma_start(out=feat32[:, 0, :], in_=feat.ap()[0:128, :])
        nc.sync.dma_start(out=feat32[:, 1, :], in_=feat.ap()[128:256, :])

        make_identity(nc, ident32[:])

        # ---------------- anchor bilinear indices (f32, whole Q) --------
        # ix = clip(((x + 1) * 0.5) * (L-1), 0, L-1); i0 = min(floor, L-2)
        nc.vector.tensor_scalar(out=ixf[:], in0=xq[:], scalar1=1.0,
                                scalar2=0.5, op0=Alu.add, op1=Alu.mult)
        nc.vector.tensor_scalar(out=ixf[:], in0=ixf[:], scalar1=float(IXSCALE),
                                scalar2=0.0, op0=Alu.mult, op1=Alu.max)
        nc.vector.tensor_scalar(out=ixf[:], in0=ixf[:], scalar1=float(IXSCALE),
                                scalar2=None, op0=Alu.min)
        ti_a = small.tile([P, G], I32)
        gt_a = small.tile([P, G], F32)
        nc.vector.tensor_copy(out=ti_a[:], in_=ixf[:])
        nc.vector.tensor_copy(out=i0fa[:], in_=ti_a[:])
        nc.vector.tensor_tensor(out=gt_a[:], in0=i0fa[:], in1=ixf[:],
                                op=Alu.is_gt)
        nc.vector.tensor_tensor(out=i0fa[:], in0=i0fa[:], in1=gt_a[:],
                                op=Alu.subtract)
        nc.vector.tensor_scalar(out=i0fa[:], in0=i0fa[:], scalar1=float(L - 2),
                                scalar2=None, op0=Alu.min)
        nc.vector.tensor_tensor(out=fraca[:], in0=ixf[:], in1=i0fa[:],
                                op=Alu.subtract)

        # ---------------- cast feat to fp16 (Scalar engine) -------------
        nc.scalar.copy(out=feat16[:, 0, :], in_=feat32[:, 0, :])
        nc.scalar.copy(out=feat16[:, 1, :], in_=feat32[:, 1, :])

        # ---------------- U = feat.T @ W1f  ([L, 64] f32) ----------------
        # 4 full-bank PSUM groups, each 8 l-blocks x 2 c-halves = 16 matmuls.
        for grp in range(4):
            psu = psU.tile([P, 8, H], F32, tag="psU", space="PSUM")
            for j in range(8):
                lb = grp * 8 + j
                for hh in range(2):
                    w_sb = w1a0_sb if hh == 0 else w1a1_sb
                    nc.tensor.matmul(
                        out=psu[:, j, :],
                        lhsT=feat16[:, hh, lb * 128:(lb + 1) * 128],
                        rhs=w_sb[:],
                        start=(j == 0 and hh == 0),
                        stop=(j == 7 and hh == 1))
            nc.scalar.copy(out=stagU[:, grp * 8:(grp + 1) * 8, :], in_=psu[:])
        nc.sync.dma_start(
            out=U_dram[:].rearrange("(t p) h -> p t h", p=P), in_=stagU[:])
        if "d_U" in dbg:
            nc.sync.dma_start(out=dbg["d_U"].ap(),
                              in_=stagU[:].rearrange("p t h -> p (t h)"))

        # ---------------- wrapA (anchor gather indices) ------------------
        # W_a[m, n] = V[16a + m%16, n]; col f = g*8 + a
        for a in range(8):
            psw = psU.tile([P, G], F32, tag="psU", space="PSUM")
            nc.tensor.matmul(out=psw[:], lhsT=sel_sb[:, a * 128:(a + 1) * 128],
                             rhs=i0fa[:], start=True, stop=True)
            dstA = AP(tensor=wrapA[:].tensor, offset=wrapA[:].offset + a,
                      ap=[wrapA[:].ap[0], [8, G]])
            nc.vector.tensor_copy(out=dstA, in_=psw[:])

        # ---------------- feat_T (fp16) via PE "transposes" ---------------
        # Normal matmuls against an fp16 identity (avoids transpose-mode and
        # fp16-in-PSUM): 16 bank groups x 4 matmuls each, f32 PSUM, cast on
        # the PSUM->SBUF copy.
        ident16 = small.tile([P, P], F16)
        nc.vector.tensor_copy(out=ident16[:], in_=ident32[:])
        for grp in range(16):
            ptt = pst.tile([P, 4, P], F32, tag="psT", space="PSUM")
            for j in range(4):
                s = grp * 2 + j // 2
                hh = j % 2
                nc.tensor.matmul(
                    out=ptt[:, j, :],
                    lhsT=feat16[:, hh, s * 128:(s + 1) * 128],
                    rhs=ident16[:],
                    start=(j == 0), stop=(j == 3))
            dstT = AP(tensor=stagT[:].tensor,
                      offset=stagT[:].offset + (grp * 2) * C,
                      ap=[stagT[:].ap[0], [C, 2], [P, 2], [1, P]])
            if grp % 2 == 0:
                nc.vector.tensor_copy(out=dstT, in_=ptt[:])
            else:
                nc.scalar.copy(out=dstT, in_=ptt[:])
            nc.sync.dma_start(
                out=feat_T[grp * 256:(grp + 1) * 256, :].rearrange(
                    "(t p) c -> p t c", p=P),
                in_=stagT[:, grp * 2:(grp + 1) * 2, :])
        if "d_featT" in dbg:
            nc.sync.dma_start(out=dbg["d_featT"].ap(),
                              in_=stagT[:].rearrange("p t c -> p (t c)"))

        nc.gpsimd.memset(h_sb[H:H + 1, :], 1.0)
        nc.gpsimd.memset(gaug[H:H + 1, :], 1.0)

        # gather sources
        gsrcU = AP(tensor=U_dram[:].tensor, offset=0,
                   ap=[[H, L - 1], [1, 2 * H]])
        gsrcT = AP(tensor=feat_T[:].tensor, offset=0,
                   ap=[[C, L - 1], [1, 2 * C]])

        # ================= chunk pipeline =================
        def front(ch):
            sl512 = [slice(ch * QC + i * 512, ch * QC + (i + 1) * 512)
                     for i in range(2)]
            gsl = slice(ch * GC, (ch + 1) * GC)

            # ---- anchor gather from U ----
            Ua = gathA.tile([P, GC, 2 * H], F32, tag="Ua")
            nc.gpsimd.dma_gather(
                out_ap=Ua[:], in_ap=gsrcU,
                idxs_ap=wrapA[:, ch * (QC // 16):(ch + 1) * (QC // 16)],
                num_idxs=QC, num_idxs_reg=QC, elem_size=2 * H, elem_step=H)

            # ---- lerp to h_pre (query-major [p, gi, 64] f32) ----
            dU = fap.tile([P, GC, H], F32, tag="dU")
            nc.vector.tensor_tensor(out=dU[:], in0=Ua[:, :, H:2 * H],
                                    in1=Ua[:, :, 0:H], op=Alu.subtract)
            fa = fap.tile([P, GC, H], F32, tag="fa")
            for gi in range(GC):
                g = ch * GC + gi
                nc.vector.scalar_tensor_tensor(
                    out=fa[:, gi, :], in0=dU[:, gi, :],
                    scalar=fraca[:, g:g + 1],
                    in1=Ua[:, gi, 0:H], op0=Alu.mult, op1=Alu.add)

            # ---- layer 1: transpose h_pre + xc/b1 contribution ----
            for half in range(2):
                ps1 = psa.tile([H, 512], F32, tag="ps1", space="PSUM")
                for j in range(4):
                    gi = half * 4 + j
                    nc.tensor.matmul(
                        out=ps1[:, j * 128:(j + 1) * 128],
                        lhsT=fa[:, gi, :], rhs=ident32[:],
                        start=(j == 0), stop=False)
                nc.tensor.matmul(out=ps1[:], lhsT=wxc3_sb[:],
                                 rhs=xc3_sb[:, sl512[half]],
                                 start=False, stop=True)
                tmp1 = fap.tile([H, 512], F32, tag="l1tmp")
                nc.scalar.copy(out=tmp1[:], in_=ps1[:])
                nc.vector.scalar_tensor_tensor(
                    out=h_sb[0:H, sl512[half]], in0=tmp1[:], scalar=0.2,
                    in1=tmp1[:], op0=Alu.mult, op1=Alu.max)

            # ---- layer 2: g = leaky(h @ (Wr+I) + br) ----
            for half in range(2):
                ps2 = psb.tile([H, 512], F32, tag="ps2", space="PSUM")
                nc.tensor.matmul(out=ps2[:], lhsT=wr1_sb[:],
                                 rhs=h_sb[:, sl512[half]],
                                 start=True, stop=True)
                tmp2 = fap.tile([H, 512], F32, tag="l2tmp")
                nc.scalar.copy(out=tmp2[:], in_=ps2[:])
                nc.vector.scalar_tensor_tensor(
                    out=gaug[0:H, sl512[half]], in0=tmp2[:], scalar=0.2,
                    in1=tmp2[:], op0=Alu.mult, op1=Alu.max)

            # ---- layer 3: out3 [p, gi, 12] ----
            out3 = sc.tile([P, GC, 12], F32, tag="out3")
            for gi in range(GC):
                g = ch * GC + gi
                ps3 = psl3.tile([P, 12], F32, tag="ps3", space="PSUM")
                nc.tensor.matmul(out=ps3[:], lhsT=gaug[:, g * 128:(g + 1) * 128],
                                 rhs=w3_sb[:], start=True, stop=True)
                nc.scalar.copy(out=out3[:, gi, :], in_=ps3[:])
            if ch == 0 and "d_out3" in dbg:
                nc.sync.dma_start(
                    out=dbg["d_out3"].ap(),
                    in_=out3[:].rearrange("p g k -> p (g k)"))

            # ---- scalar stage (per-chunk [p, GC] / [p, GC*K] f32) ----
            def softplus(dst, src_ap):
                aT = sc.tile([P, GC], F32, tag="sp_a")
                nc.scalar.activation(out=aT[:], in_=src_ap, func=Act.Abs)
                eT = sc.tile([P, GC], F32, tag="sp_e")
                nc.scalar.activation(out=eT[:], in_=aT[:], func=Act.Exp,
                                     scale=-1.0)
                lg = sc.tile([P, GC], F32, tag="sp_l")
                nc.scalar.activation(out=lg[:], in_=eT[:], func=Act.Ln,
                                     bias=1.0, scale=1.0)
                mT = sc.tile([P, GC], F32, tag="sp_m")
                nc.vector.tensor_scalar(out=mT[:], in0=src_ap, scalar1=0.0,
                                        scalar2=None, op0=Alu.max)
                nc.vector.tensor_tensor(out=dst, in0=lg[:], in1=mT[:],
                                        op=Alu.add)

            # tanh/sigmoid first (one act-table), then exp/ln ops
            res_t = sc.tile([P, GC * K], F32, tag="res")
            nc.scalar.activation(out=res_t[:], in_=out3[:, :, 2:7],
                                 func=Act.Tanh)
            gate_t = sc.tile([P, GC * K], F32, tag="gate")
            nc.scalar.activation(out=gate_t[:], in_=out3[:, :, 7:12],
                                 func=Act.Sigmoid)

            r_t = sc.tile([P, GC], F32, tag="r")
            softplus(r_t[:], out3[:, :, 0])
            nc.vector.tensor_scalar(out=r_t[:], in0=r_t[:], scalar1=0.3,
                                    scalar2=2.0, op0=Alu.add, op1=Alu.min)
            sg_t = sc.tile([P, GC], F32, tag="sg")
            softplus(sg_t[:], out3[:, :, 1])
            nc.vector.tensor_scalar(out=sg_t[:], in0=sg_t[:], scalar1=0.5,
                                    scalar2=3.0, op0=Alu.add, op1=Alu.min)
            s2 = sc.tile([P, GC], F32, tag="s2")
            nc.vector.tensor_tensor(out=s2[:], in0=sg_t[:], in1=sg_t[:],
                                    op=Alu.mult)
            nc.vector.tensor_scalar(out=s2[:], in0=s2[:], scalar1=4.0,
                                    scalar2=1e-8, op0=Alu.mult, op1=Alu.add)
            rs = sc.tile([P, GC], F32, tag="rs")
            nc.vector.reciprocal(out=rs[:], in_=s2[:])

            off_t = sc.tile([P, GC * K], F32, tag="off")
            nc.vector.tensor_tensor(out=off_t[:], in0=_bc(r_t[:], K),
                                    in1=_bc_mid(base_sb[:], GC), op=Alu.mult)
            nc.vector.scalar_tensor_tensor(out=off_t[:], in0=res_t[:],
                                           scalar=0.5, in1=off_t[:],
                                           op0=Alu.mult, op1=Alu.add)
            dix = sc.tile([P, GC * K], F32, tag="dix")
            nc.vector.scalar_tensor_tensor(out=dix[:], in0=off_t[:],
                                           scalar=float(DXSCALE),
                                           in1=_bc(xq[:, gsl], K),
                                           op0=Alu.mult, op1=Alu.add)
            nc.vector.tensor_scalar(out=dix[:], in0=dix[:], scalar1=1.0,
                                    scalar2=0.5, op0=Alu.add, op1=Alu.mult)
            nc.vector.tensor_scalar(out=dix[:], in0=dix[:],
                                    scalar1=float(IXSCALE),
                                    scalar2=0.0, op0=Alu.mult, op1=Alu.max)
            nc.vector.tensor_scalar(out=dix[:], in0=dix[:],
                                    scalar1=float(IXSCALE),
                                    scalar2=None, op0=Alu.min)
            fracd = sc.tile([P, GC * K], F32, tag="fracd")
            i0fd = sc.tile([P, GC * K], F32, tag="i0fd")
            ti_d = sc.tile([P, GC * K], I32, tag="ti_d")
            nc.vector.tensor_copy(out=ti_d[:], in_=dix[:])
            nc.vector.tensor_copy(out=i0fd[:], in_=ti_d[:])
            gt_d = sc.tile([P, GC * K], F32, tag="gt_d")
            nc.vector.tensor_tensor(out=gt_d[:], in0=i0fd[:], in1=dix[:],
                                    op=Alu.is_gt)
            nc.vector.tensor_tensor(out=i0fd[:], in0=i0fd[:], in1=gt_d[:],
                                    op=Alu.subtract)
            nc.vector.tensor_scalar(out=i0fd[:], in0=i0fd[:],
                                    scalar1=float(L - 2),
                                    scalar2=None, op0=Alu.min)
            nc.vector.tensor_tensor(out=fracd[:], in0=dix[:], in1=i0fd[:],
                                    op=Alu.subtract)

            o2 = sc.tile([P, GC * K], F32, tag="o2")
            nc.vector.tensor_tensor(out=o2[:], in0=off_t[:], in1=off_t[:],
                                    op=Alu.mult)
            nc.vector.tensor_tensor(out=o2[:], in0=o2[:], in1=_bc(rs[:], K),
                                    op=Alu.mult)
            w_t = sc.tile([P, GC * K], F32, tag="w")
            nc.scalar.activation(out=w_t[:], in_=o2[:], func=Act.Exp,
                                 scale=-0.5)
            nc.vector.tensor_tensor(out=w_t[:], in0=w_t[:], in1=gate_t[:],
                                    op=Alu.mult)
            wsum = sc.tile([P, GC], F32, tag="wsum")
            w_v = w_t[:].rearrange("p (g k) -> p g k", k=K)
            nc.vector.tensor_reduce(out=wsum[:], in_=w_v,
                                    axis=mybir.AxisListType.X, op=Alu.add)
            nc.vector.tensor_scalar(out=wsum[:], in0=wsum[:], scalar1=1e-8,
                                    scalar2=None, op0=Alu.add)
            rn = sc.tile([P, GC], F32, tag="rn")
            nc.vector.reciprocal(out=rn[:], in_=wsum[:])
            wn = sc.tile([P, GC * K], F32, tag="wn")
            nc.vector.tensor_tensor(out=wn[:], in0=w_t[:], in1=_bc(rn[:], K),
                                    op=Alu.mult)
            c1 = sc.tile([P, GC * K], F32, tag="c1")
            nc.vector.tensor_tensor(out=c1[:], in0=wn[:], in1=fracd[:],
                                    op=Alu.mult)
            c0 = sc.tile([P, GC * K], F32, tag="c0")
            nc.vector.tensor_tensor(out=c0[:], in0=wn[:], in1=c1[:],
                                    op=Alu.subtract)
            if ch == 0 and "d_i0fd" in dbg:
                nc.sync.dma_start(out=dbg["d_i0fd"].ap(), in_=i0fd[:])
                nc.sync.dma_start(out=dbg["d_c0"].ap(), in_=c0[:])
                nc.sync.dma_start(out=dbg["d_c1"].ap(), in_=c1[:])

            # ---- wrapped deform indices, per half-chunk ----
            # j = k*512 + q_local; wrapDh[b, k, g_local*8 + a]
            wrapDs = []
            for hf in range(2):
                wrapDh = sc.tile([P, K, QH // 16], I16, tag=f"wrapD{hf}")
                vsl = i0fd[:, hf * GH * K:(hf + 1) * GH * K]
                for a in range(8):
                    psw = psb.tile([P, GH * K], F32, tag="ps2", space="PSUM")
                    nc.tensor.matmul(out=psw[:],
                                     lhsT=sel_sb[:, a * 128:(a + 1) * 128],
                                     rhs=vsl, start=True, stop=True)
                    dstD = AP(tensor=wrapDh[:].tensor,
                              offset=wrapDh[:].offset + a,
                              ap=[wrapDh[:].ap[0], [QH // 16, K], [8, GH]])
                    srcD = AP(tensor=psw[:].tensor, offset=psw[:].offset,
                              ap=[psw[:].ap[0], [1, K], [K, GH]])
                    nc.vector.tensor_copy(out=dstD, in_=srcD)
                wrapDs.append(wrapDh)
            return c0, c1, wrapDs

        def deform(ch, c0, c1, wrapDs):
            ob = obp.tile([P, GC, C], F16, tag="ob")
            for hf in range(2):
                Gd = gathD.tile([P, K * GH, 2 * C], F16, tag="Gd")
                # HW limit: <=1024 idxs per dma_gather call; j = k*512 + q
                for (k0, k1) in ((0, 2), (2, 4), (4, 5)):
                    nc.gpsimd.dma_gather(
                        out_ap=Gd[:, k0 * GH:k1 * GH, :], in_ap=gsrcT,
                        idxs_ap=wrapDs[hf][:, k0:k1, :],
                        num_idxs=(k1 - k0) * QH,
                        num_idxs_reg=(k1 - k0) * QH,
                        elem_size=2 * C, elem_step=C)
                for gi in range(GH):
                    gl = hf * GH + gi
                    acc = sc.tile([P, C], F16, tag="acc")
                    for k in range(K):
                        col = k * GH + gi
                        s0 = c0[:, gl * K + k:gl * K + k + 1]
                        s1 = c1[:, gl * K + k:gl * K + k + 1]
                        tgt = ob[:, gl, :] if k == K - 1 else acc[:]
                        if k == 0:
                            nc.vector.tensor_scalar(
                                out=acc[:], in0=Gd[:, col, 0:C],
                                scalar1=s0, scalar2=None, op0=Alu.mult)
                        else:
                            nc.vector.scalar_tensor_tensor(
                                out=acc[:], in0=Gd[:, col, 0:C],
                                scalar=s0, in1=acc[:],
                                op0=Alu.mult, op1=Alu.add)
                        nc.vector.scalar_tensor_tensor(
                            out=tgt, in0=Gd[:, col, C:2 * C],
                            scalar=s1, in1=acc[:],
                            op0=Alu.mult, op1=Alu.add)
            nc.sync.dma_start(
                out=out.ap()[ch * QC:(ch + 1) * QC, :].rearrange(
                    "(g p) c -> p g c", p=P),
                in_=ob[:])

        # software pipeline: front(ch+1) issued before deform(ch)
        pend = front(0)
        for ch in range(1, NCH):
            nxt = front(ch)
            deform(ch - 1, *pend)
            pend = nxt
        deform(NCH - 1, *pend)

        if "d_h" in dbg:
            nc.sync.dma_start(out=dbg["d_h"].ap(), in_=h_sb[:])
            nc.sync.dma_start(out=dbg["d_g"].ap(), in_=gaug[:])


_PROGRAM = None


def _get_program():
    global _PROGRAM
    if _PROGRAM is None:
        _PROGRAM = build_program()
    return _PROGRAM


def make_in_maps(feat_1d, coords_1d, cell_1d, W1, b1, Wr, br, W3, b3):
    """Build the 8 per-core input dicts from full inputs."""
    f32, f16 = np.float32, np.float16
    W1 = np.asarray(W1, f32)
    wr1aug = np.concatenate(
        [np.asarray(Wr, f32) + np.eye(H, dtype=f32),
         np.asarray(br, f32).reshape(1, H)], axis=0).astype(f16)
    w3aug = np.concatenate([np.asarray(W3, f32),
                            np.asarray(b3, f32).reshape(1, 12)],
                           axis=0).astype(f16)
    wxc3 = np.concatenate([W1[256:258], np.asarray(b1, f32).reshape(1, H)],
                          axis=0).astype(f16)
    base = np.array([-2.0, -1.0, 0.0, 1.0, 2.0], f32)
    base128 = np.broadcast_to(base, (P, K)).copy()
    sel = np.zeros((P, 8, 128), f32)
    for a in range(8):
        for m in range(128):
            sel[16 * a + m % 16, a, m] = 1.0
    shared = {
        "w1a0": np.ascontiguousarray(W1[0:128]).astype(f16),
        "w1a1": np.ascontiguousarray(W1[128:256]).astype(f16),
        "wxc3": wxc3,
        "wr1aug": wr1aug,
        "w3aug": w3aug,
        "base128": base128,
        "sel8": sel.reshape(P, 8 * 128),
    }
    in_maps = []
    for core in range(NCORES):
        b = core // 2
        s = core % 2
        sl = slice(s * Q, (s + 1) * Q)
        cq = np.ascontiguousarray(np.asarray(coords_1d[b, sl, 0], f32))
        cl = np.ascontiguousarray(np.asarray(cell_1d[b, sl, 0], f32))
        xc3h = np.stack([cq, cl, np.ones_like(cq)], axis=0).astype(f16)
        in_maps.append({
            "feat": np.ascontiguousarray(np.asarray(feat_1d[b], f32)),
            "coords": cq,
            "xc3h": xc3h,
            **shared,
        })
    return in_maps


def kernel(feat_1d, coords_1d, cell_1d, W1, b1, Wr, br, W3, b3):
    from concourse.bass_utils import run_bass_kernel_spmd
    nc = _get_program()
    in_maps = make_in_maps(feat_1d, coords_1d, cell_1d, W1, b1, Wr, br, W3, b3)
    res = run_bass_kernel_spmd(nc, in_maps, core_ids=list(range(NCORES)))
    outf = np.zeros((B, N, C), np.float32)
    for core in range(NCORES):
        b = core // 2
        s = core % 2
        outf[b, s * Q:(s + 1) * Q, :] = np.asarray(
            res.results[core]["out"], dtype=np.float32)
    return outf


# revision 15
# speedup vs baseline: 1.4593x; 1.0761x over previous
"""Trainium2 Bass kernel for nn_DeformableDynamicGather1D (v2).

Sharding: 8 cores = 4 batches x 2 query-halves; per core feat [256, 4096],
Q=4096 queries.

Key ideas vs v1 baseline:
  1. Anchor path folds layer-1 of the router MLP into a precomputed table
     U[l, h] = sum_c feat[c, l] * W1[c, h]  ([L, 64], f32 in DRAM, computed
     on PE straight from the channel-major feat with NO transpose).  The
     anchor gather then fetches 512B row-pairs of U instead of 2KB row-pairs
     of feat (4x less traffic) and the expensive 256-channel query->channel
     transposes disappear: lerp produces h_pre query-major [q, 64], a cheap
     PE transpose + a small accumulated matmul adds the coords/cell/b1
     contribution.
  2. feat_T for the deform gather is stored in fp16: deform gather traffic
     drops 40MB -> 20MB per core; DVE tap-combine runs on packed fp16
     (2x_1P mode eligible).  Output is written fp16 and cast to f32 on host.
  3. MLP runs in fp16 on the PE (1 cycle/row vs 4 for fp32) with biases
     folded into augmented weights (ones rows).
  4. Chunked software pipeline: queries processed in 4 chunks of 1024;
     chunk ch+1's anchor/MLP/scalar front is issued before chunk ch's
     deform gather+combine, so GpSimd gather transfers, DVE tap-FMAs and
     PE front work overlap across chunks.
  5. PSUM matmul groups are batched per 2KB bank (8 transposes or 8 U
     matmuls per accumulation group) to amortize PSUM->SBUF copies.
"""
import os
import sys

for _p in ("/opt/trn_rl_repo", "/root/.axon_site/_ro/trn_rl_repo"):
    if os.path.isdir(_p) and _p not in sys.path:
        sys.path.append(_p)

import numpy as np
import concourse.bass as bass
import concourse.bacc as bacc
import concourse.tile as tile
from concourse import mybir
from concourse.bass import AP
from concourse.masks import make_identity

F32 = mybir.dt.float32
F16 = mybir.dt.float16
I16 = mybir.dt.int16
I32 = mybir.dt.int32
Act = mybir.ActivationFunctionType
Alu = mybir.AluOpType

P = 128          # partitions
G = 32           # q = g*128 + p
Q = P * G        # 4096 queries per core
C = 256          # channels
L = 4096         # feat length
H = 64           # hidden
K = 5            # taps
NCORES = 8
B, N = 4, 8192   # full problem
NCH = 4          # query chunks
GC = G // NCH    # 8 g-columns per chunk
QC = P * GC      # 1024 queries per chunk
GH = GC // 2     # 4 g-columns per half-chunk
QH = P * GH      # 512 queries per half-chunk

IXSCALE = np.float32(float(L - 1))          # 4095
DXSCALE = np.float32(2.0 / max(L - 1, 1))   # reference scale_x

DEBUG_DUMPS = False


def _bc(ap2d: AP, extra: int) -> AP:
    """Broadcast a [p, n] AP to [p, n, extra] with stride-0 inner dim."""
    return AP(tensor=ap2d.tensor, offset=ap2d.offset,
              ap=[*ap2d.ap, [0, extra]])


def _bc_mid(ap2d: AP, mid: int) -> AP:
    """Broadcast a [p, n] AP to [p, mid, n] with stride-0 middle dim."""
    return AP(tensor=ap2d.tensor, offset=ap2d.offset,
              ap=[ap2d.ap[0], [0, mid], ap2d.ap[1]])


def build_program():
    nc = bacc.Bacc("TRN2", target_bir_lowering=False, debug=False,
                   num_devices=NCORES)

    feat = nc.dram_tensor("feat", [C, L], F32, kind="ExternalInput")
    coords = nc.dram_tensor("coords", [Q], F32, kind="ExternalInput")
    xc3h = nc.dram_tensor("xc3h", [3, Q], F16, kind="ExternalInput")
    w1a0 = nc.dram_tensor("w1a0", [128, H], F16, kind="ExternalInput")
    w1a1 = nc.dram_tensor("w1a1", [128, H], F16, kind="ExternalInput")
    wxc3 = nc.dram_tensor("wxc3", [3, H], F16, kind="ExternalInput")
    wr1aug = nc.dram_tensor("wr1aug", [H + 1, H], F16, kind="ExternalInput")
    w3aug = nc.dram_tensor("w3aug", [H + 1, 12], F16, kind="ExternalInput")
    base128 = nc.dram_tensor("base128", [P, K], F32, kind="ExternalInput")
    sel8 = nc.dram_tensor("sel8", [P, 8 * 128], F32, kind="ExternalInput")
    out = nc.dram_tensor("out", [Q, C], F16, kind="ExternalOutput")

    dbg = {}
    if DEBUG_DUMPS:
        dbg = {
            "d_U": nc.dram_tensor("d_U", [P, G * H], F32, kind="ExternalOutput"),
            "d_featT": nc.dram_tensor("d_featT", [P, G * C], F16, kind="ExternalOutput"),
            "d_h": nc.dram_tensor("d_h", [H + 1, Q], F16, kind="ExternalOutput"),
            "d_g": nc.dram_tensor("d_g", [H + 1, Q], F16, kind="ExternalOutput"),
            "d_out3": nc.dram_tensor("d_out3", [P, GC * 12], F32, kind="ExternalOutput"),
            "d_i0fd": nc.dram_tensor("d_i0fd", [P, GC * K], F32, kind="ExternalOutput"),
            "d_c0": nc.dram_tensor("d_c0", [P, GC * K], F32, kind="ExternalOutput"),
            "d_c1": nc.dram_tensor("d_c1", [P, GC * K], F32, kind="ExternalOutput"),
        }

    with tile.TileContext(nc) as tc:
        _body(nc, tc, feat, coords, xc3h, w1a0, w1a1, wxc3, wr1aug,
              w3aug, base128, sel8, out, dbg)
    nc.compile()
    return nc


def _body(nc, tc, feat, coords, xc3h, w1a0, w1a1, wxc3, wr1aug,
          w3aug, base128, sel8, out, dbg=None):
    dbg = dbg or {}
    import contextlib
    ctx = contextlib.ExitStack()
    with ctx:
        persist = ctx.enter_context(tc.tile_pool(name="persist", bufs=1))
        small = ctx.enter_context(tc.tile_pool(name="small", bufs=1))
        gathA = ctx.enter_context(tc.tile_pool(name="gathA", bufs=2))
        gathD = ctx.enter_context(tc.tile_pool(name="gathD", bufs=2))
        fap = ctx.enter_context(tc.tile_pool(name="fap", bufs=2))
        obp = ctx.enter_context(tc.tile_pool(name="obp", bufs=2))
        sc = ctx.enter_context(tc.tile_pool(name="scal", bufs=2))
        pst = ctx.enter_context(tc.tile_pool(name="pst", bufs=1, space="PSUM"))
        psU = ctx.enter_context(tc.tile_pool(name="psU", bufs=2, space="PSUM"))
        psa = ctx.enter_context(tc.tile_pool(name="psa", bufs=2, space="PSUM"))
        psb = ctx.enter_context(tc.tile_pool(name="psb", bufs=2, space="PSUM"))
        psl3 = ctx.enter_context(tc.tile_pool(name="psl3", bufs=1, space="PSUM"))
        dram = ctx.enter_context(tc.tile_pool(name="dram", bufs=1, space="DRAM"))

        # ---------------- persistent tiles ----------------
        feat32 = persist.tile([P, 2, L], F32)     # c-halves of feat
        feat16 = persist.tile([P, 2, L], F16)
        stagT = persist.tile([P, G, C], F16)      # feat_T staging (t-major)
        stagU = persist.tile([P, G, H], F32)      # U staging
        h_sb = persist.tile([H + 1, Q], F16)      # row H = 1.0
        gaug = persist.tile([H + 1, Q], F16)      # row H = 1.0
        xq = persist.tile([P, G], F32)
        ixf = persist.tile([P, G], F32)
        fraca = persist.tile([P, G], F32)
        i0fa = persist.tile([P, G], F32)

        feat_T = dram.tile([L, C], F16)
        U_dram = dram.tile([L, H], F32)

        # ---------------- weights / constants ----------------
        w1a0_sb = small.tile([128, H], F16)
        w1a1_sb = small.tile([128, H], F16)
        wxc3_sb = small.tile([3, H], F16)
        wr1_sb = small.tile([H + 1, H], F16)
        w3_sb = small.tile([H + 1, 12], F16)
        base_sb = small.tile([P, K], F32)
        sel_sb = small.tile([P, 8 * 128], F32)
        xc3_sb = small.tile([3, Q], F16)
        ident32 = small.tile([P, P], F32)
        wrapA = small.tile([P, Q // 16], I16)

        for dst, src in ((w1a0_sb, w1a0), (w1a1_sb, w1a1), (wxc3_sb, wxc3),
                         (wr1_sb, wr1aug), (w3_sb, w3aug),
                         (base_sb, base128), (sel_sb, sel8), (xc3_sb, xc3h)):
            nc.sync.dma_start(out=dst[:], in_=src.ap())
        nc.sync.dma_start(
            out=xq[:],
            in_=AP(tensor=coords.ap().tensor, offset=0, ap=[[1, P], [P, G]]))
        nc.sync.dma_start(out=feat32[:, 0, :], in_=feat.ap()[0:128, :])
        nc.sync.dma_start(out=feat32[:, 1, :], in_=feat.ap()[128:256, :])

        make_identity(nc, ident32[:])

        # ---------------- anchor bilinear indices (f32, whole Q) --------
        # ix = clip(((x + 1) * 0.5) * (L-1), 0, L-1); i0 = min(floor, L-2)
        nc.vector.tensor_scalar(out=ixf[:], in0=xq[:], scalar1=1.0,
                                scalar2=0.5, op0=Alu.add, op1=Alu.mult)
        nc.vector.tensor_scalar(out=ixf[:], in0=ixf[:], scalar1=float(IXSCALE),
                                scalar2=0.0, op0=Alu.mult, op1=Alu.max)
        nc.vector.tensor_scalar(out=ixf[:], in0=ixf[:], scalar1=float(IXSCALE),
                                scalar2=None, op0=Alu.min)
        ti_a = small.tile([P, G], I32)
        gt_a = small.tile([P, G], F32)
        nc.vector.tensor_copy(out=ti_a[:], in_=ixf[:])
        nc.vector.tensor_copy(out=i0fa[:], in_=ti_a[:])
        nc.vector.tensor_tensor(out=gt_a[:], in0=i0fa[:], in1=ixf[:],
                                op=Alu.is_gt)
        nc.vector.tensor_tensor(out=i0fa[:], in0=i0fa[:], in1=gt_a[:],
                                op=Alu.subtract)
        nc.vector.tensor_scalar(out=i0fa[:], in0=i0fa[:], scalar1=float(L - 2),
                                scalar2=None, op0=Alu.min)
        nc.vector.tensor_tensor(out=fraca[:], in0=ixf[:], in1=i0fa[:],
                                op=Alu.subtract)

        # ---------------- cast feat to fp16 (Scalar engine) -------------
        nc.scalar.copy(out=feat16[:, 0, :], in_=feat32[:, 0, :])
        nc.scalar.copy(out=feat16[:, 1, :], in_=feat32[:, 1, :])

        # ---------------- U = feat.T @ W1f  ([L, 64] f32) ----------------
        # 4 full-bank PSUM groups, each 8 l-blocks x 2 c-halves = 16 matmuls.
        for grp in range(4):
            psu = psU.tile([P, 8, H], F32, tag="psU", space="PSUM")
            for j in range(8):
                lb = grp * 8 + j
                for hh in range(2):
                    w_sb = w1a0_sb if hh == 0 else w1a1_sb
                    nc.tensor.matmul(
                        out=psu[:, j, :],
                        lhsT=feat16[:, hh, lb * 128:(lb + 1) * 128],
                        rhs=w_sb[:],
                        start=(j == 0 and hh == 0),
                        stop=(j == 7 and hh == 1))
            nc.scalar.copy(out=stagU[:, grp * 8:(grp + 1) * 8, :], in_=psu[:])
        nc.sync.dma_start(
            out=U_dram[:].rearrange("(t p) h -> p t h", p=P), in_=stagU[:])
        if "d_U" in dbg:
            nc.sync.dma_start(out=dbg["d_U"].ap(),
                              in_=stagU[:].rearrange("p t h -> p (t h)"))

        # ---------------- wrapA (anchor gather indices) ------------------
        # W_a[m, n] = V[16a + m%16, n]; col f = g*8 + a
        for a in range(8):
            psw = psU.tile([P, G], F32, tag="psU", space="PSUM")
            nc.tensor.matmul(out=psw[:], lhsT=sel_sb[:, a * 128:(a + 1) * 128],
                             rhs=i0fa[:], start=True, stop=True)
            dstA = AP(tensor=wrapA[:].tensor, offset=wrapA[:].offset + a,
                      ap=[wrapA[:].ap[0], [8, G]])
            nc.vector.tensor_copy(out=dstA, in_=psw[:])

        # ---------------- feat_T (fp16) via PE "transposes" ---------------
        # Normal matmuls against an fp16 identity (avoids transpose-mode and
        # fp16-in-PSUM): 16 bank groups x 4 matmuls each, f32 PSUM, cast on
        # the PSUM->SBUF copy.
        ident16 = small.tile([P, P], F16)
        nc.vector.tensor_copy(out=ident16[:], in_=ident32[:])
        for grp in range(16):
            ptt = pst.tile([P, 4, P], F32, tag="psT", space="PSUM")
            for j in range(4):
                s = grp * 2 + j // 2
                hh = j % 2
                nc.tensor.matmul(
                    out=ptt[:, j, :],
                    lhsT=feat16[:, hh, s * 128:(s + 1) * 128],
                    rhs=ident16[:],
                    start=(j == 0), stop=(j == 3))
            dstT = AP(tensor=stagT[:].tensor,
                      offset=stagT[:].offset + (grp * 2) * C,
                      ap=[stagT[:].ap[0], [C, 2], [P, 2], [1, P]])
            if grp % 2 == 0:
                nc.vector.tensor_copy(out=dstT, in_=ptt[:])
            else:
                nc.scalar.copy(out=dstT, in_=ptt[:])
            nc.sync.dma_start(
                out=feat_T[grp * 256:(grp + 1) * 256, :].rearrange(
                    "(t p) c -> p t c", p=P),
                in_=stagT[:, grp * 2:(grp + 1) * 2, :])
        if "d_featT" in dbg:
            nc.sync.dma_start(out=dbg["d_featT"].ap(),
                              in_=stagT[:].rearrange("p t c -> p (t c)"))

        nc.gpsimd.memset(h_sb[H:H + 1, :], 1.0)
        nc.gpsimd.memset(gaug[H:H + 1, :], 1.0)

        # gather sources
        gsrcU = AP(tensor=U_dram[:].tensor, offset=0,
                   ap=[[H, L - 1], [1, 2 * H]])
        gsrcT = AP(tensor=feat_T[:].tensor, offset=0,
                   ap=[[C, L - 1], [1, 2 * C]])

        # ================= chunk pipeline =================
        def front(ch):
            sl512 = [slice(ch * QC + i * 512, ch * QC + (i + 1) * 512)
                     for i in range(2)]
            gsl = slice(ch * GC, (ch + 1) * GC)

            # ---- anchor gather from U ----
            Ua = gathA.tile([P, GC, 2 * H], F32, tag="Ua")
            nc.gpsimd.dma_gather(
                out_ap=Ua[:], in_ap=gsrcU,
                idxs_ap=wrapA[:, ch * (QC // 16):(ch + 1) * (QC // 16)],
                num_idxs=QC, num_idxs_reg=QC, elem_size=2 * H, elem_step=H)

            # ---- lerp to h_pre (query-major [p, gi, 64] f32) ----
            dU = fap.tile([P, GC, H], F32, tag="dU")
            nc.vector.tensor_tensor(out=dU[:], in0=Ua[:, :, H:2 * H],
                                    in1=Ua[:, :, 0:H], op=Alu.subtract)
            fa = fap.tile([P, GC, H], F32, tag="fa")
            for gi in range(GC):
                g = ch * GC + gi
                nc.vector.scalar_tensor_tensor(
                    out=fa[:, gi, :], in0=dU[:, gi, :],
                    scalar=fraca[:, g:g + 1],
                    in1=Ua[:, gi, 0:H], op0=Alu.mult, op1=Alu.add)

            # ---- layer 1: transpose h_pre + xc/b1 contribution ----
            for half in range(2):
                ps1 = psa.tile([H, 512], F32, tag="ps1", space="PSUM")
                for j in range(4):
                    gi = half * 4 + j
                    nc.tensor.matmul(
                        out=ps1[:, j * 128:(j + 1) * 128],
                        lhsT=fa[:, gi, :], rhs=ident32[:],
                        start=(j == 0), stop=False)
                nc.tensor.matmul(out=ps1[:], lhsT=wxc3_sb[:],
                                 rhs=xc3_sb[:, sl512[half]],
                                 start=False, stop=True)
                tmp1 = fap.tile([H, 512], F32, tag="l1tmp")
                nc.scalar.copy(out=tmp1[:], in_=ps1[:])
                nc.vector.scalar_tensor_tensor(
                    out=h_sb[0:H, sl512[half]], in0=tmp1[:], scalar=0.2,
                    in1=tmp1[:], op0=Alu.mult, op1=Alu.max)

            # ---- layer 2: g = leaky(h @ (Wr+I) + br) ----
            for half in range(2):
                ps2 = psb.tile([H, 512], F32, tag="ps2", space="PSUM")
                nc.tensor.matmul(out=ps2[:], lhsT=wr1_sb[:],
                                 rhs=h_sb[:, sl512[half]],
                                 start=True, stop=True)
                tmp2 = fap.tile([H, 512], F32, tag="l2tmp")
                nc.scalar.copy(out=tmp2[:], in_=ps2[:])
                nc.vector.scalar_tensor_tensor(
                    out=gaug[0:H, sl512[half]], in0=tmp2[:], scalar=0.2,
                    in1=tmp2[:], op0=Alu.mult, op1=Alu.max)

            # ---- layer 3: out3 [p, gi, 12] ----
            out3 = sc.tile([P, GC, 12], F32, tag="out3")
            for gi in range(GC):
                g = ch * GC + gi
                ps3 = psl3.tile([P, 12], F32, tag="ps3", space="PSUM")
                nc.tensor.matmul(out=ps3[:], lhsT=gaug[:, g * 128:(g + 1) * 128],
                                 rhs=w3_sb[:], start=True, stop=True)
                nc.scalar.copy(out=out3[:, gi, :], in_=ps3[:])
            if ch == 0 and "d_out3" in dbg:
                nc.sync.dma_start(
                    out=dbg["d_out3"].ap(),
                    in_=out3[:].rearrange("p g k -> p (g k)"))

            # ---- scalar stage (per-chunk [p, GC] / [p, GC*K] f32) ----
            # softplus(x) = max(x,0) + ln(1+u), u = exp(-|x|); ln(1+u) via
            # atanh series: 2z(1 + z^2/3 + z^4/5), z = u/(2+u) (|err|<6e-5).
            # Keeps ACT funcs to {Copy, Abs, Exp, Tanh} = one act table.
            def softplus(dst, src_ap):
                aT = sc.tile([P, GC], F32, tag="sp_a")
                nc.scalar.activation(out=aT[:], in_=src_ap, func=Act.Abs)
                uT = sc.tile([P, GC], F32, tag="sp_e")
                nc.scalar.activation(out=uT[:], in_=aT[:], func=Act.Exp,
                                     scale=-1.0)
                tT = sc.tile([P, GC], F32, tag="sp_t")
                nc.vector.tensor_scalar(out=tT[:], in0=uT[:], scalar1=2.0,
                                        scalar2=None, op0=Alu.add)
                rT = sc.tile([P, GC], F32, tag="sp_r")
                nc.vector.reciprocal(out=rT[:], in_=tT[:])
                zT = sc.tile([P, GC], F32, tag="sp_z")
                nc.vector.tensor_tensor(out=zT[:], in0=uT[:], in1=rT[:],
                                        op=Alu.mult)
                z2 = sc.tile([P, GC], F32, tag="sp_z2")
                nc.vector.tensor_tensor(out=z2[:], in0=zT[:], in1=zT[:],
                                        op=Alu.mult)
                h1 = sc.tile([P, GC], F32, tag="sp_h1")
                nc.vector.tensor_scalar(out=h1[:], in0=z2[:], scalar1=0.4,
                                        scalar2=2.0 / 3.0, op0=Alu.mult,
                                        op1=Alu.add)
                nc.vector.tensor_tensor(out=h1[:], in0=h1[:], in1=z2[:],
                                        op=Alu.mult)
                nc.vector.tensor_scalar(out=h1[:], in0=h1[:], scalar1=2.0,
                                        scalar2=None, op0=Alu.add)
                nc.vector.tensor_tensor(out=h1[:], in0=h1[:], in1=zT[:],
                                        op=Alu.mult)
                mT = sc.tile([P, GC], F32, tag="sp_m")
                nc.vector.tensor_scalar(out=mT[:], in0=src_ap, scalar1=0.0,
                                        scalar2=None, op0=Alu.max)
                nc.vector.tensor_tensor(out=dst, in0=h1[:], in1=mT[:],
                                        op=Alu.add)

            res_t = sc.tile([P, GC * K], F32, tag="res")
            nc.scalar.activation(out=res_t[:], in_=out3[:, :, 2:7],
                                 func=Act.Tanh)
            # sigmoid(x) = 0.5 + 0.5*tanh(x/2) (avoids the sigmoid act table)
            gate_t = sc.tile([P, GC * K], F32, tag="gate")
            nc.scalar.activation(out=gate_t[:], in_=out3[:, :, 7:12],
                                 func=Act.Tanh, scale=0.5)
            nc.vector.tensor_scalar(out=gate_t[:], in0=gate_t[:], scalar1=0.5,
                                    scalar2=0.5, op0=Alu.mult, op1=Alu.add)

            r_t = sc.tile([P, GC], F32, tag="r")
            softplus(r_t[:], out3[:, :, 0])
            nc.vector.tensor_scalar(out=r_t[:], in0=r_t[:], scalar1=0.3,
                                    scalar2=2.0, op0=Alu.add, op1=Alu.min)
            sg_t = sc.tile([P, GC], F32, tag="sg")
            softplus(sg_t[:], out3[:, :, 1])
            nc.vector.tensor_scalar(out=sg_t[:], in0=sg_t[:], scalar1=0.5,
                                    scalar2=3.0, op0=Alu.add, op1=Alu.min)
            s2 = sc.tile([P, GC], F32, tag="s2")
            nc.vector.tensor_tensor(out=s2[:], in0=sg_t[:], in1=sg_t[:],
                                    op=Alu.mult)
            nc.vector.tensor_scalar(out=s2[:], in0=s2[:], scalar1=4.0,
                                    scalar2=1e-8, op0=Alu.mult, op1=Alu.add)
            rs = sc.tile([P, GC], F32, tag="rs")
            nc.vector.reciprocal(out=rs[:], in_=s2[:])

            off_t = sc.tile([P, GC * K], F32, tag="off")
            nc.vector.tensor_tensor(out=off_t[:], in0=_bc(r_t[:], K),
                                    in1=_bc_mid(base_sb[:], GC), op=Alu.mult)
            nc.vector.scalar_tensor_tensor(out=off_t[:], in0=res_t[:],
                                           scalar=0.5, in1=off_t[:],
                                           op0=Alu.mult, op1=Alu.add)
            dix = sc.tile([P, GC * K], F32, tag="dix")
            nc.vector.scalar_tensor_tensor(out=dix[:], in0=off_t[:],
                                           scalar=float(DXSCALE),
                                           in1=_bc(xq[:, gsl], K),
                                           op0=Alu.mult, op1=Alu.add)
            nc.vector.tensor_scalar(out=dix[:], in0=dix[:], scalar1=1.0,
                                    scalar2=0.5, op0=Alu.add, op1=Alu.mult)
            nc.vector.tensor_scalar(out=dix[:], in0=dix[:],
                                    scalar1=float(IXSCALE),
                                    scalar2=0.0, op0=Alu.mult, op1=Alu.max)
            nc.vector.tensor_scalar(out=dix[:], in0=dix[:],
                                    scalar1=float(IXSCALE),
                                    scalar2=None, op0=Alu.min)
            fracd = sc.tile([P, GC * K], F32, tag="fracd")
            i0fd = sc.tile([P, GC * K], F32, tag="i0fd")
            ti_d = sc.tile([P, GC * K], I32, tag="ti_d")
            nc.vector.tensor_copy(out=ti_d[:], in_=dix[:])
            nc.vector.tensor_copy(out=i0fd[:], in_=ti_d[:])
            gt_d = sc.tile([P, GC * K], F32, tag="gt_d")
            nc.vector.tensor_tensor(out=gt_d[:], in0=i0fd[:], in1=dix[:],
                                    op=Alu.is_gt)
            nc.vector.tensor_tensor(out=i0fd[:], in0=i0fd[:], in1=gt_d[:],
                                    op=Alu.subtract)
            nc.vector.tensor_scalar(out=i0fd[:], in0=i0fd[:],
                                    scalar1=float(L - 2),
                                    scalar2=None, op0=Alu.min)
            nc.vector.tensor_tensor(out=fracd[:], in0=dix[:], in1=i0fd[:],
                                    op=Alu.subtract)

            o2 = sc.tile([P, GC * K], F32, tag="o2")
            nc.vector.tensor_tensor(out=o2[:], in0=off_t[:], in1=off_t[:],
                                    op=Alu.mult)
            nc.vector.tensor_tensor(out=o2[:], in0=o2[:], in1=_bc(rs[:], K),
                                    op=Alu.mult)
            w_t = sc.tile([P, GC * K], F32, tag="w")
            nc.scalar.activation(out=w_t[:], in_=o2[:], func=Act.Exp,
                                 scale=-0.5)
            nc.vector.tensor_tensor(out=w_t[:], in0=w_t[:], in1=gate_t[:],
                                    op=Alu.mult)
            wsum = sc.tile([P, GC], F32, tag="wsum")
            w_v = w_t[:].rearrange("p (g k) -> p g k", k=K)
            nc.vector.tensor_reduce(out=wsum[:], in_=w_v,
                                    axis=mybir.AxisListType.X, op=Alu.add)
            nc.vector.tensor_scalar(out=wsum[:], in0=wsum[:], scalar1=1e-8,
                                    scalar2=None, op0=Alu.add)
            rn = sc.tile([P, GC], F32, tag="rn")
            nc.vector.reciprocal(out=rn[:], in_=wsum[:])
            wn = sc.tile([P, GC * K], F32, tag="wn")
            nc.vector.tensor_tensor(out=wn[:], in0=w_t[:], in1=_bc(rn[:], K),
                                    op=Alu.mult)
            c1 = sc.tile([P, GC * K], F32, tag="c1")
            nc.vector.tensor_tensor(out=c1[:], in0=wn[:], in1=fracd[:],
                                    op=Alu.mult)
            c0 = sc.tile([P, GC * K], F32, tag="c0")
            nc.vector.tensor_tensor(out=c0[:], in0=wn[:], in1=c1[:],
                                    op=Alu.subtract)
            if ch == 0 and "d_i0fd" in dbg:
                nc.sync.dma_start(out=dbg["d_i0fd"].ap(), in_=i0fd[:])
                nc.sync.dma_start(out=dbg["d_c0"].ap(), in_=c0[:])
                nc.sync.dma_start(out=dbg["d_c1"].ap(), in_=c1[:])

            # ---- interleaved fp16 coefficients for the diag-matmul combine:
            # cfi16[p, gi, k, half] = c{half}[p, gi*K + k]
            cfi16 = sc.tile([P, GC, K, 2], F16, tag="cfi")
            nc.vector.tensor_copy(out=cfi16[:, :, :, 0],
                                  in_=c0[:].rearrange("p (g k) -> p g k", k=K))
            nc.vector.tensor_copy(out=cfi16[:, :, :, 1],
                                  in_=c1[:].rearrange("p (g k) -> p g k", k=K))

            # ---- wrapped deform indices, per half-chunk ----
            # j = k*512 + q_local; wrapDh[b, k, g_local*8 + a]
            wrapDs = []
            for hf in range(2):
                wrapDh = sc.tile([P, K, QH // 16], I16, tag=f"wrapD{hf}")
                vsl = i0fd[:, hf * GH * K:(hf + 1) * GH * K]
                for a in range(8):
                    psw = psb.tile([P, GH * K], F32, tag="ps2", space="PSUM")
                    nc.tensor.matmul(out=psw[:],
                                     lhsT=sel_sb[:, a * 128:(a + 1) * 128],
                                     rhs=vsl, start=True, stop=True)
                    dstD = AP(tensor=wrapDh[:].tensor,
                              offset=wrapDh[:].offset + a,
                              ap=[wrapDh[:].ap[0], [QH // 16, K], [8, GH]])
                    srcD = AP(tensor=psw[:].tensor, offset=psw[:].offset,
                              ap=[psw[:].ap[0], [1, K], [K, GH]])
                    nc.vector.tensor_copy(out=dstD, in_=srcD)
                wrapDs.append(wrapDh)
            return cfi16, wrapDs

        def deform(ch, cfi16, wrapDs):
            # combine taps on the PE: out[q, c] = sum_j diag(c_j) @ F_j,
            # PSUM-accumulated over the 10 (tap, half) pairs per 128-query
            # block.  diag tiles are built on DVE as ident16 * coef-broadcast.
            ob = obp.tile([P, GC, C], F16, tag="ob")
            for hf in range(2):
                Gd = gathD.tile([P, K * GH, 2 * C], F16, tag="Gd")
                # HW limit: <=1024 idxs per dma_gather call; j = k*512 + q
                for (k0, k1) in ((0, 2), (2, 4), (4, 5)):
                    nc.gpsimd.dma_gather(
                        out_ap=Gd[:, k0 * GH:k1 * GH, :], in_ap=gsrcT,
                        idxs_ap=wrapDs[hf][:, k0:k1, :],
                        num_idxs=(k1 - k0) * QH,
                        num_idxs_reg=(k1 - k0) * QH,
                        elem_size=2 * C, elem_step=C)
                for gi in range(GH):
                    gl = hf * GH + gi
                    diag = sc.tile([P, 2 * K, P], F16, tag="diag")
                    cin = AP(tensor=cfi16[:].tensor,
                             offset=cfi16[:].offset + gl * 2 * K,
                             ap=[cfi16[:].ap[0], [1, 2 * K], [0, P]])
                    nc.vector.tensor_tensor(out=diag[:],
                                            in0=_bc_mid(ident16[:], 2 * K),
                                            in1=cin, op=Alu.mult)
                    psc = psU.tile([P, C], F32, tag="psU", space="PSUM")
                    for k in range(K):
                        for half in range(2):
                            j = k * 2 + half
                            nc.tensor.matmul(
                                out=psc[:],
                                lhsT=diag[:, j, :],
                                rhs=Gd[:, k * GH + gi, half * C:(half + 1) * C],
                                start=(j == 0), stop=(j == 2 * K - 1))
                    nc.scalar.copy(out=ob[:, gl, :], in_=psc[:])
            nc.sync.dma_start(
                out=out.ap()[ch * QC:(ch + 1) * QC, :].rearrange(
                    "(g p) c -> p g c", p=P),
                in_=ob[:])

        # software pipeline: front(ch+1) issued before deform(ch)
        pend = front(0)
        for ch in range(1, NCH):
            nxt = front(ch)
            deform(ch - 1, *pend)
            pend = nxt
        deform(NCH - 1, *pend)

        if "d_h" in dbg:
            nc.sync.dma_start(out=dbg["d_h"].ap(), in_=h_sb[:])
            nc.sync.dma_start(out=dbg["d_g"].ap(), in_=gaug[:])


_PROGRAM = None


def _get_program():
    global _PROGRAM
    if _PROGRAM is None:
        _PROGRAM = build_program()
    return _PROGRAM


def make_in_maps(feat_1d, coords_1d, cell_1d, W1, b1, Wr, br, W3, b3):
    """Build the 8 per-core input dicts from full inputs."""
    f32, f16 = np.float32, np.float16
    W1 = np.asarray(W1, f32)
    wr1aug = np.concatenate(
        [np.asarray(Wr, f32) + np.eye(H, dtype=f32),
         np.asarray(br, f32).reshape(1, H)], axis=0).astype(f16)
    w3aug = np.concatenate([np.asarray(W3, f32),
                            np.asarray(b3, f32).reshape(1, 12)],
                           axis=0).astype(f16)
    wxc3 = np.concatenate([W1[256:258], np.asarray(b1, f32).reshape(1, H)],
                          axis=0).astype(f16)
    base = np.array([-2.0, -1.0, 0.0, 1.0, 2.0], f32)
    base128 = np.broadcast_to(base, (P, K)).copy()
    sel = np.zeros((P, 8, 128), f32)
    for a in range(8):
        for m in range(128):
            sel[16 * a + m % 16, a, m] = 1.0
    shared = {
        "w1a0": np.ascontiguousarray(W1[0:128]).astype(f16),
        "w1a1": np.ascontiguousarray(W1[128:256]).astype(f16),
        "wxc3": wxc3,
        "wr1aug": wr1aug,
        "w3aug": w3aug,
        "base128": base128,
        "sel8": sel.reshape(P, 8 * 128),
    }
    in_maps = []
    for core in range(NCORES):
        b = core // 2
        s = core % 2
        sl = slice(s * Q, (s + 1) * Q)
        cq = np.ascontiguousarray(np.asarray(coords_1d[b, sl, 0], f32))
        cl = np.ascontiguousarray(np.asarray(cell_1d[b, sl, 0], f32))
        xc3h = np.stack([cq, cl, np.ones_like(cq)], axis=0).astype(f16)
        in_maps.append({
            "feat": np.ascontiguousarray(np.asarray(feat_1d[b], f32)),
            "coords": cq,
            "xc3h": xc3h,
            **shared,
        })
    return in_maps


def kernel(feat_1d, coords_1d, cell_1d, W1, b1, Wr, br, W3, b3):
    from concourse.bass_utils import run_bass_kernel_spmd
    nc = _get_program()
    in_maps = make_in_maps(feat_1d, coords_1d, cell_1d, W1, b1, Wr, br, W3, b3)
    res = run_bass_kernel_spmd(nc, in_maps, core_ids=list(range(NCORES)))
    outf = np.zeros((B, N, C), np.float32)
    for core in range(NCORES):
        b = core // 2
        s = core % 2
        outf[b, s * Q:(s + 1) * Q, :] = np.asarray(
            res.results[core]["out"], dtype=np.float32)
    return outf
